# revision 1
# baseline (speedup 1.0000x reference)
"""Conv-MoE (top-2 of 8 experts, 3x3 SAME conv 128->128 on 56x56) on 8 TRN2 cores.

Strategy:
  - Gate (GAP -> logits -> top-2 -> softmax) computed on host: it is tiny and
    determines the dispatch.
  - Conv is linear in the weights, so the weighted two-expert combine folds
    into a single per-image kernel:  sum_e w_be * conv(x_b, W_e)
    == conv(x_b, sum_e w_be * W_e).  One 3x3 conv per image.
  - Data parallel: 4 images per NeuronCore.  Each image is one padded
    (58x58) SBUF tile; the 3x3 conv is 9 accumulating fp32r (TF32) matmuls
    per PSUM bank, 7 banks of 8 output rows (N=448 columns) per image.
  - Host packs [padded image | 9 tap weights] into one contiguous blob per
    image so each image needs exactly one input DMA (also keeps the
    fp32/fp32r matmul 1-wait ISA limit satisfiable).
"""

import sys

import numpy as np

for _p in ("/opt/trn_rl_repo/concourse", "/opt/trn_rl_repo"):
    if _p not in sys.path:
        sys.path.insert(0, _p)

import concourse.mybir as mybir
import concourse.tile as tile
from concourse import bacc
from concourse.bass_utils import run_bass_kernel_spmd

N_CORES = 8
B, CIN, H, W = 32, 128, 56, 56
E, COUT = 8, 128
PER = B // N_CORES            # images per core
HP, WP = H + 2, W + 2         # padded image
IMG = HP * WP                 # 3364
WCOL = 9 * COUT               # 1152
BLOB = IMG + WCOL
RG, RPG = 7, 8                # 7 row-groups of 8 rows
N = RPG * W                   # 448 matmul columns


def _build():
    nc = bacc.Bacc(None, target_bir_lowering=False, debug=False)
    blob = nc.dram_tensor("blob", [PER, 128, BLOB], mybir.dt.float32,
                          kind="ExternalInput")
    out = nc.dram_tensor("out", [PER, 128, H * W], mybir.dt.float32,
                         kind="ExternalOutput")

    with tile.TileContext(nc) as tc:
        with (
            tc.tile_pool(name="sb", bufs=2) as sb,
            tc.tile_pool(name="ps", bufs=1, space="PSUM") as ps,
        ):
            for b in range(PER):
                bt = sb.tile([128, BLOB], mybir.dt.float32r, tag="bt",
                             name=f"bt{b}")
                nc.gpsimd.dma_start(bt[:, :], blob[b, :, :])  # casts to fp32r
                xp = bt[:, :IMG].rearrange("p (h w) -> p h w", h=HP)

                pts = [ps.tile([128, N], mybir.dt.float32, tag=f"ps{rg}",
                               name=f"pt{b}_{rg}") for rg in range(RG)]
                for t in range(9):
                    dy, dx = divmod(t, 3)
                    lhsT = bt[:, IMG + t * COUT: IMG + (t + 1) * COUT]
                    for rg in range(RG):
                        rhs = xp[:, rg * RPG + dy: rg * RPG + dy + RPG,
                                 dx: dx + W]
                        nc.tensor.matmul(pts[rg][:, :], lhsT, rhs,
                                         start=(t == 0), stop=(t == 8))

                ot = sb.tile([128, H * W], mybir.dt.float32, tag="ot",
                             name=f"ot{b}")
                for rg in range(RG):
                    dst = ot[:, rg * N:(rg + 1) * N]
                    if rg % 2 == 0:
                        nc.scalar.copy(dst, pts[rg][:, :])
                    else:
                        nc.vector.tensor_copy(dst, pts[rg][:, :])
                nc.sync.dma_start(out[b, :, :], ot[:, :])

    nc.compile()
    return nc


_NC = None


def _gate(x, gate_w, gate_b, top_k):
    """Replicates the reference gate in float32 numpy."""
    k = int(top_k)
    gap = x.reshape(B, CIN, H * W).mean(axis=2, dtype=np.float32)
    logits = (gap @ gate_w.T + gate_b).astype(np.float32)     # [B, E]
    # jax.lax.top_k: k largest, ties broken toward lower index.
    idx = np.argsort(-logits, axis=1, kind="stable")[:, :k]   # [B, k]
    vals = np.take_along_axis(logits, idx, axis=1)
    m = vals.max(axis=1, keepdims=True)
    ex = np.exp(vals - m)
    sm = (ex / ex.sum(axis=1, keepdims=True)).astype(np.float32)
    weights = np.zeros((B, E), np.float32)
    np.put_along_axis(weights, idx, sm, axis=1)
    return weights, idx, sm


def kernel(x, conv_w, gate_w, gate_b, top_k):
    global _NC
    x = np.ascontiguousarray(np.asarray(x, dtype=np.float32))
    conv_w = np.asarray(conv_w, dtype=np.float32)
    gate_w = np.asarray(gate_w, dtype=np.float32)
    gate_b = np.asarray(gate_b, dtype=np.float32)

    weights, idx, sm = _gate(x, gate_w, gate_b, top_k)

    # Per-image combined weights, tap-major with Cin on partitions:
    # wtap[b, ci, t*128 + o] = sum_j sm[b,j] * conv_w[idx[b,j], o, ci, dy, dx]
    k = idx.shape[1]
    comb = np.zeros((B, COUT, CIN, 3, 3), np.float32)
    for j in range(k):
        comb += sm[:, j, None, None, None, None] * conv_w[idx[:, j]]
    # [B, Cout, Cin, 3, 3] -> [B, Cin, 9, Cout] -> [B, Cin, WCOL]
    wtap = comb.transpose(0, 2, 3, 4, 1).reshape(B, CIN, 9 * COUT)

    # Blobs: [B, 128, BLOB] = [padded image | taps]
    blobs = np.zeros((B, CIN, BLOB), np.float32)
    xp = blobs[:, :, :IMG].reshape(B, CIN, HP, WP)
    xp[:, :, 1:1 + H, 1:1 + W] = x
    blobs[:, :, IMG:] = wtap

    if _NC is None:
        _NC = _build()
    in_maps = [{"blob": blobs[c * PER:(c + 1) * PER]} for c in range(N_CORES)]
    res = run_bass_kernel_spmd(_NC, in_maps, core_ids=list(range(N_CORES)))
    out = np.concatenate([r["out"] for r in res.results], axis=0)
    out = out.reshape(B, COUT, H, W)
    return out, weights


# revision 2
# speedup vs baseline: 1.0767x; 1.0767x over previous
"""Conv-MoE (top-2 of 8 experts, 3x3 SAME conv 128->128 on 56x56) on 8 TRN2 cores.

Strategy:
  - Gate (GAP -> logits -> top-2 -> softmax) computed on host: it is tiny and
    determines the dispatch.
  - Conv is linear in the weights, so the weighted two-expert combine folds
    into a single per-image kernel:  sum_e w_be * conv(x_b, W_e)
    == conv(x_b, sum_e w_be * W_e).  One 3x3 conv per image.
  - Data parallel: 4 images per NeuronCore.  Each image is one padded
    (58x58) SBUF tile; the 3x3 conv is 9 accumulating fp32r (TF32) matmuls
    per PSUM bank, 7 banks of 8 output rows (N=448 columns) per image.
  - Host packs [9 tap weights | padded image] into one contiguous blob per
    image, loaded in two HWDGE DMA chunks so the first row-groups can start
    while the rest of the image is still in flight (also keeps the
    fp32/fp32r matmul 1-wait ISA limit satisfiable).
"""

import sys

import numpy as np

for _p in ("/opt/trn_rl_repo/concourse", "/opt/trn_rl_repo"):
    if _p not in sys.path:
        sys.path.insert(0, _p)

import concourse.mybir as mybir
import concourse.tile as tile
from concourse import bacc
from concourse.bass_utils import run_bass_kernel_spmd

N_CORES = 8
B, CIN, H, W = 32, 128, 56, 56
E, COUT = 8, 128
PER = B // N_CORES            # images per core
HP, WP = H + 2, W + 2         # padded image
IMG = HP * WP                 # 3364
WCOL = 9 * COUT               # 1152
BLOB = WCOL + IMG             # weights first, then padded image
RG, RPG = 7, 8                # 7 row-groups of 8 rows
N = RPG * W                   # 448 matmul columns
# chunk split: weights + padded rows 0..28 | rows 29..57
SPLIT_ROW = 29
CHUNK1 = WCOL + SPLIT_ROW * WP


def _build():
    nc = bacc.Bacc(None, target_bir_lowering=False, debug=False)
    blob = nc.dram_tensor("blob", [PER, 128, BLOB], mybir.dt.float32r,
                          kind="ExternalInput")
    out = nc.dram_tensor("out", [PER, 128, H * W], mybir.dt.float32,
                         kind="ExternalOutput")

    with tile.TileContext(nc) as tc:
        with (
            tc.tile_pool(name="sb", bufs=2) as sb,
            tc.tile_pool(name="ps", bufs=1, space="PSUM") as ps,
        ):
            for b in range(PER):
                bt = sb.tile([128, BLOB], mybir.dt.float32r, tag="bt",
                             name=f"bt{b}")
                nc.sync.dma_start(bt[:, :CHUNK1], blob[b, :, :CHUNK1])
                nc.sync.dma_start(bt[:, CHUNK1:], blob[b, :, CHUNK1:])
                xp = bt[:, WCOL:].rearrange("p (h w) -> p h w", h=HP)

                ot = sb.tile([128, H * W], mybir.dt.float32, tag="ot",
                             name=f"ot{b}")
                for rg in range(RG):
                    pt = ps.tile([128, N], mybir.dt.float32, tag=f"ps{rg}",
                                 name=f"pt{b}_{rg}")
                    for t in range(9):
                        dy, dx = divmod(t, 3)
                        lhsT = bt[:, t * COUT:(t + 1) * COUT]
                        rhs = xp[:, rg * RPG + dy: rg * RPG + dy + RPG,
                                 dx: dx + W]
                        nc.tensor.matmul(pt[:, :], lhsT, rhs,
                                         start=(t == 0), stop=(t == 8))
                    dst = ot[:, rg * N:(rg + 1) * N]
                    if rg % 2 == 0:
                        nc.scalar.copy(dst, pt[:, :])
                    else:
                        nc.vector.tensor_copy(dst, pt[:, :])
                    nc.sync.dma_start(out[b, :, rg * N:(rg + 1) * N], dst)

    nc.compile()
    return nc


_NC = None


def _gate(x, gate_w, gate_b, top_k):
    """Replicates the reference gate in float32 numpy."""
    k = int(top_k)
    gap = x.reshape(B, CIN, H * W).mean(axis=2, dtype=np.float32)
    logits = (gap @ gate_w.T + gate_b).astype(np.float32)     # [B, E]
    # jax.lax.top_k: k largest, ties broken toward lower index.
    idx = np.argsort(-logits, axis=1, kind="stable")[:, :k]   # [B, k]
    vals = np.take_along_axis(logits, idx, axis=1)
    m = vals.max(axis=1, keepdims=True)
    ex = np.exp(vals - m)
    sm = (ex / ex.sum(axis=1, keepdims=True)).astype(np.float32)
    weights = np.zeros((B, E), np.float32)
    np.put_along_axis(weights, idx, sm, axis=1)
    return weights, idx, sm


def _blobs(x, conv_w, idx, sm):
    """[B, 128, BLOB] host blobs: [tap weights | padded image]."""
    k = idx.shape[1]
    comb = np.zeros((B, COUT, CIN, 3, 3), np.float32)
    for j in range(k):
        comb += sm[:, j, None, None, None, None] * conv_w[idx[:, j]]
    # wtap[b, ci, t*COUT + o] = comb[b, o, ci, dy, dx]
    wtap = comb.transpose(0, 2, 3, 4, 1).reshape(B, CIN, WCOL)
    blobs = np.zeros((B, CIN, BLOB), np.float32)
    blobs[:, :, :WCOL] = wtap
    xp = blobs[:, :, WCOL:].reshape(B, CIN, HP, WP)
    xp[:, :, 1:1 + H, 1:1 + W] = x
    return blobs


def kernel(x, conv_w, gate_w, gate_b, top_k):
    global _NC
    x = np.ascontiguousarray(np.asarray(x, dtype=np.float32))
    conv_w = np.asarray(conv_w, dtype=np.float32)
    gate_w = np.asarray(gate_w, dtype=np.float32)
    gate_b = np.asarray(gate_b, dtype=np.float32)

    weights, idx, sm = _gate(x, gate_w, gate_b, top_k)
    blobs = _blobs(x, conv_w, idx, sm)

    if _NC is None:
        _NC = _build()
    in_maps = [{"blob": blobs[c * PER:(c + 1) * PER]} for c in range(N_CORES)]
    res = run_bass_kernel_spmd(_NC, in_maps, core_ids=list(range(N_CORES)))
    out = np.concatenate([r["out"] for r in res.results], axis=0)
    out = out.reshape(B, COUT, H, W)
    return out, weights
